# revision 9
# baseline (speedup 1.0000x reference)
"""Bass/Trainium2 kernel for masked dot-product attention.

Math (per batch b):
  scores = q @ k^T / sqrt(D)
  masked positions (j >= valid[i]) replaced by 1e-6 (NOT -inf)
  weights = softmax(scores, axis=-1);  out = weights @ v

Strategy:
  - Shard batch dim B=16 across 8 cores (2 batches/core), SPMD program.
  - Host-side: sort rows of each batch by valid[i] (argsort), so the mask
    becomes a monotone staircase; gather q rows, scatter output rows back.
  - Device: compute S^T tiles [j=128, i<=512] via PE (fp32r), exp on ACT
    (scale=1/sqrt(D) folded in), zero masked positions of E on DVE only in
    staircase-boundary tiles, accumulate O^T = V''^T E^T on PE with a ones
    column appended to V to get the softmax denominator for free.
  - Fully-masked (i-range, j-tile) pairs are skipped entirely; their exact
    contribution exp(1e-6)*(suffix sums of v) is added analytically via a
    host-precomputed correction matrix multiplied by identity into the same
    PSUM accumulator.
  - Final: PE-transpose O^T back to [i, d], divide by denominator (DVE),
    DMA out.
"""

import ml_dtypes
import numpy as np

import concourse.bass as bass
import concourse.tile as tile
import concourse.mybir as mybir
from concourse import bacc
from concourse.bass_utils import run_bass_kernel_spmd
from concourse.masks import make_identity

B, N, D = 16, 2048, 64
NCORES = 8
NB = B // NCORES          # batches per core
IW = 512                  # i-range width (moving dim of both matmuls)
NI = N // IW              # 4 i-ranges
JW = 128                  # j-tile width (partition dim of S^T)
NJ = N // JW              # 16 j-tiles
DV = D + 1                # V with ones column appended

f32 = mybir.dt.float32
f32r = mybir.dt.float32r
bf16 = mybir.dt.float16  # fp16: 2-byte speed with 10-bit mantissa


def _classify(t_sorted):
    """Per (i-range, j-tile) role, uniform across all batches (SPMD).

    Returns cls[r][tau] = None (skip) or (lo, mlo, mhi): compute columns
    [lo, IW) (cols < lo are fully masked for every batch and covered by the
    analytic correction); apply the mask multiply on [mlo, mhi).
    """
    cls = []
    for r in range(NI):
        row = []
        tw = t_sorted[:, r * IW:(r + 1) * IW]  # [B, IW] sorted ascending
        for tau in range(NJ):
            jlo, jhi = JW * tau, JW * (tau + 1)
            n_le = (tw <= jlo).sum(axis=1)   # rows fully masked in this tile
            n_lt = (tw < jhi).sum(axis=1)    # rows with any mask in this tile
            if (n_le == IW).all():
                row.append(None)
            else:
                lo = int(n_le.min()) & ~15   # 16-elem aligned for PE APs
                mhi = int(n_lt.max())
                row.append((lo, lo, max(mhi, lo)))
        cls.append(row)
    return cls


def _build_program(cls):
    nc = bacc.Bacc("TRN2", target_bir_lowering=False, debug=False)

    qs = nc.dram_tensor("qs", [NB, N, D], bf16, kind="ExternalInput").ap()
    kk = nc.dram_tensor("kk", [NB, N, D], bf16, kind="ExternalInput").ap()
    vv = nc.dram_tensor("vv", [NB, N, D], bf16, kind="ExternalInput").ap()
    corr = nc.dram_tensor("corr", [NB, DV, N], f32r, kind="ExternalInput").ap()
    trow = nc.dram_tensor("trow", [NB, 1, N], f32, kind="ExternalInput").ap()
    out = nc.dram_tensor("out", [NB, N, D], f32, kind="ExternalOutput").ap()

    scale = 1.0 / np.sqrt(np.float32(D))

    with tile.TileContext(nc, trace_sim=False) as tc:
        with (
            tc.tile_pool(name="consts", bufs=1) as consts,
            tc.tile_pool(name="sb_in", bufs=2) as sb_in,
            tc.tile_pool(name="sb_T", bufs=2) as sb_T,
            tc.tile_pool(name="sb_e", bufs=3) as sb_e,
            tc.tile_pool(name="sb_m", bufs=3) as sb_m,
            tc.tile_pool(name="sb_f", bufs=2) as sb_f,
            tc.tile_pool(name="sb_o", bufs=2) as sb_o,
            tc.tile_pool(name="sb_z", bufs=4) as sb_z,
            tc.tile_pool(name="ps_t", bufs=1, space="PSUM") as ps_t,
            tc.tile_pool(name="ps_s", bufs=2, space="PSUM") as ps_s,
            tc.tile_pool(name="ps_acc", bufs=2, space="PSUM") as ps_acc,
            tc.tile_pool(name="ps_g", bufs=1, space="PSUM") as ps_g,
        ):
            ident = consts.tile([128, 128], f32)
            make_identity(nc, ident)
            identr = consts.tile([128, 128], f32r)
            nc.vector.tensor_copy(identr, ident)
            identb = consts.tile([128, 128], bf16)
            nc.vector.tensor_copy(identb, ident)
            onesc = consts.tile([128, NJ], f32)
            nc.vector.memset(onesc, 1.0)
            piota_i = consts.tile([128, IW], mybir.dt.int32)
            nc.gpsimd.iota(piota_i, pattern=[[0, IW]], channel_multiplier=1)
            piota = consts.tile([128, IW], f32)
            nc.vector.tensor_copy(piota, piota_i)

            for bi in range(NB):
                qnat = sb_in.tile([128, NJ, D], bf16, tag="qnat")
                knat = sb_in.tile([128, NJ, D], bf16, tag="knat")
                vnat = sb_in.tile([128, NJ, D], bf16, tag="vnat")
                nc.sync.dma_start(
                    out=qnat, in_=qs[bi].rearrange("(g p) d -> p g d", p=128))
                nc.sync.dma_start(
                    out=knat, in_=kk[bi].rearrange("(g p) d -> p g d", p=128))
                nc.sync.dma_start(
                    out=vnat, in_=vv[bi].rearrange("(g p) d -> p g d", p=128))
                corr_sb = sb_T.tile([DV, N], f32r, tag="corr")
                nc.sync.dma_start(out=corr_sb, in_=corr[bi])
                trow_sb = sb_T.tile([1, N], f32, tag="trow")
                nc.sync.dma_start(out=trow_sb, in_=trow[bi])

                tb = sb_T.tile([128, N], f32, tag="tb")
                nc.gpsimd.partition_broadcast(tb, trow_sb)

                qT = sb_T.tile([D, N], bf16, tag="qT")
                kT = sb_T.tile([D, N], bf16, tag="kT")
                for src, dst in ((qnat, qT), (knat, kT)):
                    for g4 in range(NJ // 4):
                        pt = ps_t.tile([D, 512], bf16, tag="pt")
                        for u in range(4):
                            nc.tensor.transpose(
                                pt[:, bass.ts(u, 128)],
                                src[:, g4 * 4 + u, :], identb)
                        nc.vector.tensor_copy(
                            dst[:, bass.ts(g4, 512)], pt)

                vw = sb_in.tile([128, NJ, DV], bf16, tag="vw")
                for g in range(NJ):
                    nc.vector.tensor_copy(vw[:, g, 0:D], vnat[:, g, :])
                nc.vector.tensor_copy(
                    vw[:, :, D:DV],
                    onesc.rearrange("p (g o) -> p g o", o=1))

                for r in range(NI):
                    computed = [tau for tau in range(NJ)
                                if cls[r][tau] is not None]
                    pairs = [computed[i:i + 2]
                             for i in range(0, len(computed), 2)]
                    pacc = ps_acc.tile([DV, IW], f32)
                    nc.tensor.matmul(
                        pacc,
                        identr[0:DV, 0:DV],
                        corr_sb[:, bass.ts(r, IW)],
                        start=True, stop=(len(computed) == 0))
                    for pair in pairs:
                        plo = min(cls[r][tau][0] for tau in pair)
                        ps = ps_s.tile([128, 2, IW], f32)
                        e = sb_e.tile([128, 2, IW], bf16)
                        for h, tau in enumerate(pair):
                            nc.tensor.matmul(
                                ps[:, h, plo:IW],
                                kT[:, bass.ts(tau, 128)],
                                qT[:, r * IW + plo: (r + 1) * IW],
                                start=True, stop=True)
                        nh = len(pair)
                        nc.scalar.activation(
                            e[:, 0:nh, plo:IW], ps[:, 0:nh, plo:IW],
                            mybir.ActivationFunctionType.Exp,
                            scale=float(scale))
                        for h, tau in enumerate(pair):
                            lo, mlo, mhi = cls[r][tau]
                            if mhi > mlo:
                                m = sb_m.tile([128, IW], bf16)
                                # m[p, i] = ((t_i - 128*tau) > p) -> valid
                                nc.vector.scalar_tensor_tensor(
                                    out=m[:, mlo:mhi],
                                    in0=tb[:, r * IW + mlo: r * IW + mhi],
                                    scalar=float(JW * tau),
                                    in1=piota[:, mlo:mhi],
                                    op0=mybir.AluOpType.subtract,
                                    op1=mybir.AluOpType.is_gt)
                                nc.vector.tensor_mul(
                                    e[:, h, mlo:mhi], e[:, h, mlo:mhi],
                                    m[:, mlo:mhi])
                            nc.tensor.matmul(
                                pacc[:, lo:IW],
                                vw[:, tau, :],
                                e[:, h, lo:IW],
                                start=False,
                                stop=(tau == computed[-1]))

                    fsb = sb_f.tile([DV, IW], f32)
                    nc.vector.tensor_copy(fsb, pacc)
                    outt = sb_o.tile([128, IW // 128, D], f32)
                    for s in range(IW // 128):
                        pg = ps_g.tile([128, DV], f32)
                        nc.tensor.transpose(
                            pg, fsb[:, bass.ts(s, 128)], ident[0:DV, 0:DV])
                        zinv = sb_z.tile([128, 1], f32)
                        nc.vector.reciprocal(zinv, pg[:, D:DV])
                        nc.vector.tensor_scalar_mul(
                            outt[:, s, :], pg[:, 0:D], zinv)
                    nc.sync.dma_start(
                        out=out[bi, bass.ts(r, IW)].rearrange(
                            "(s p) d -> p s d", p=128),
                        in_=outt)
    nc.compile()
    return nc


LAST = {}


def kernel(q, k, v, valid, _trace=False):
    q = np.ascontiguousarray(np.asarray(q, dtype=np.float32))
    k = np.ascontiguousarray(np.asarray(k, dtype=np.float32))
    v = np.ascontiguousarray(np.asarray(v, dtype=np.float32))
    t = np.clip(np.asarray(valid).astype(np.int64), 0, N)

    perm = np.argsort(t, axis=1, kind="stable")
    t_s = np.take_along_axis(t, perm, axis=1)
    q_s = np.take_along_axis(q, perm[..., None], axis=1)

    e6 = float(np.exp(np.float32(1e-6)))
    # suffix sums of v in f64: ss[b, t] = sum_{j >= t} v[b, j]
    ss = np.zeros((B, N + 1, D), np.float64)
    ss[:, :-1] = np.cumsum(v[:, ::-1, :].astype(np.float64), axis=1)[:, ::-1, :]
    ssg = np.take_along_axis(ss, t_s[..., None], axis=1)       # [B, N, D]
    cnt = (N - t_s)[..., None].astype(np.float64)              # [B, N, 1]
    corr = np.concatenate([ssg, cnt], axis=2) * e6             # [B, N, DV]
    corrT = np.ascontiguousarray(
        np.swapaxes(corr, 1, 2)).astype(np.float32)            # [B, DV, N]
    trow = t_s.astype(np.float32)[:, None, :]                  # [B, 1, N]

    cls = _classify(t_s)
    nc = _build_program(cls)

    in_maps = []
    for c in range(NCORES):
        sl = slice(c * NB, (c + 1) * NB)
        in_maps.append({
            "qs": np.ascontiguousarray(q_s[sl]).astype(np.float16),
            "kk": np.ascontiguousarray(k[sl]).astype(np.float16),
            "vv": np.ascontiguousarray(v[sl]).astype(np.float16),
            "corr": corrT[sl],
            "trow": np.ascontiguousarray(trow[sl]),
        })
    res = run_bass_kernel_spmd(nc, in_maps, list(range(NCORES)),
                               trace=_trace)
    LAST["res"] = res
    LAST["nc"] = nc

    out = np.empty((B, N, D), np.float32)
    for c in range(NCORES):
        o = res.results[c]["out"]
        for bi in range(NB):
            b = c * NB + bi
            out[b, perm[b]] = o[bi]
    return out
